# revision 9
# baseline (speedup 1.0000x reference)
import os
import sys

sys.path.insert(0, "/opt/trn_rl_repo")

import numpy as np
import concourse.bass as bass
import concourse.mybir as mybir
import concourse.tile as tile
import concourse.tile_sem_assignment as tsa
from concourse import bass_utils
from concourse.vector_clock import ScopedClock, VectorClock

# Two HWDGE lanes: even-issued DMAs -> DMAHW0 ("A"), odd -> DMAHW1 ("B").
tsa.NUM_HWDGE_SEMS = 2


def _chunked_drain_and_barrier(self, tick_clock, wait_clock):
    # Final SP drain caps at 1 sem wait on core_v3; emit one drain per sem.
    gc = tick_clock.global_clock
    n = tsa.N_PROCS
    vals = [gc[p] for p in range(n)]
    nonzero = [p for p in range(n) if vals[p] > 0]
    for i in range(max(len(nonzero), 1)):
        group = set(nonzero[i : i + 1])
        sub = [vals[p] if p in group else 0 for p in range(n)]
        d = self.nc.sync.drain()
        wait_clock.add_sem_waits(d.ins, ScopedClock({None: VectorClock(sub)}))
    self.nc.all_engine_barrier()
    assert self.sems is not None
    popped = self.nc._tile_sem_poison_stack.pop()
    assert popped is self._sem_poison
    self.nc.clear_and_free_semaphores(list(self.sems.allocated().values()))
    self.nc.all_engine_barrier()


tile.TileContext._drain_and_barrier = _chunked_drain_and_barrier

P = 128          # SBUF partitions
NB = 9           # row blocks per image
SL = 1024        # slab width (1022 interior cols + 2 ghost cols)
W = NB * SL      # 9216
NI = 1022        # interior rows/cols
RB = 126         # interior rows per block (last block: 14)
NIT = 11         # Jacobi iterations (reference: 1 + scan(10))
HALF = 511       # half-slab matmul/STT width (cols 1..511, 512..1022)
HB = W // 2      # lane A/B column split for init loads
H = 1.0 / 1023.0


def _legalize_waits(nc):
    # CoreV3 caps most opcodes at 1 sem wait. Split extras onto no-op
    # waiters inserted just before the capped instruction (queues are
    # in-order, so blocking semantics are identical).
    seen = set()
    blocks = []
    for b in nc.bb_map.values():
        bb = b.bb
        if id(bb) not in seen:
            seen.add(id(bb))
            blocks.append(bb)
    for bb in blocks:
        il = list(bb.instructions)
        out = []
        for inst in il:
            si = getattr(inst, "sync_info", None)
            ws = list(si.on_wait) if si is not None and si.on_wait else []
            if len(ws) > 1:
                for w in ws[:-1]:
                    h = nc.engines[inst.engine].nop()
                    ni = h.ins if not hasattr(h, "opcode") else h
                    tail = nc.cur_bb.bb.instructions
                    assert tail[-1] is ni
                    tail.pop()
                    ni.sync_info = mybir.SyncInfo(on_wait=[w], on_update=[])
                    out.append(ni)
                inst.sync_info = mybir.SyncInfo(
                    on_wait=[ws[-1]], on_update=list(si.on_update or [])
                )
            out.append(inst)
        bb.instructions = out


def _build_program():
    nc = bass.Bass("TRN2", num_devices=1)
    f32 = mybir.dt.float32
    f32r = mybir.dt.float32r
    tg_ap = nc.dram_tensor("tg", [P, 512], f32, kind="ExternalInput").ap()
    y_ap = nc.dram_tensor("yp", [P, W], f32, kind="ExternalInput").ap()
    cg_ap = nc.dram_tensor("cgp", [P, W], f32, kind="ExternalInput").ap()
    o_ap = nc.dram_tensor("o", [NI, NI], f32, kind="ExternalOutput").ap()

    with tile.TileContext(nc) as tc:
        with tc.tile_pool(name="sb", bufs=1) as pool, tc.tile_pool(
            name="ps", bufs=8, space="PSUM"
        ) as psum:
            TG = pool.tile([P, 512], f32r)
            YB = pool.tile([P, W], f32r)
            CGB = pool.tile([P, W], f32r)
            TH = pool.tile([P, W], f32r)
            mwa = pool.tile([32, 4], f32r)
            mwb = pool.tile([32, 4], f32r)
            mra = pool.tile([32, 4], f32r)
            mrb = pool.tile([32, 4], f32r)
            mrd = pool.tile([32, 4], f32r)
            dscr = pool.tile([1, 1], f32r)

            # --- init loads (ACT-issued; even->laneA, odd->laneB) ---
            nc.scalar.dma_start(out=TG[:], in_=tg_ap.bitcast(f32r))              # 0 A
            nc.scalar.dma_start(out=dscr[:], in_=tg_ap[0:1, 0:1].bitcast(f32r))  # 1 B
            nc.scalar.dma_start(out=YB[:, 0:HB], in_=y_ap[:, 0:HB].bitcast(f32r))    # 2 A
            nc.scalar.dma_start(out=YB[:, HB:W], in_=y_ap[:, HB:W].bitcast(f32r))    # 3 B
            nc.scalar.dma_start(out=CGB[:, 0:HB], in_=cg_ap[:, 0:HB].bitcast(f32r))  # 4 A
            nc.scalar.dma_start(out=CGB[:, HB:W], in_=cg_ap[:, HB:W].bitcast(f32r))  # 5 B

            add = mybir.AluOpType.add
            mult = mybir.AluOpType.mult

            for k in range(NIT):
                last = k == NIT - 1
                # DVE mules: absorb lane A (dn ghosts / cg init) and lane B
                # (up ghosts / cg init) ticks into DVE stream history.
                nc.vector.tensor_copy(out=mwa[:], in_=CGB[96:128, 0:4])
                nc.vector.tensor_copy(out=mwb[:], in_=CGB[0:32, HB : HB + 4])
                # Horizontal neighbor sums per slab on Pool, overlapping the
                # DVE STT chain across the iteration boundary.
                for b in range(NB):
                    nc.gpsimd.tensor_tensor(
                        out=TH[:, b * SL + 1 : b * SL + 1023],
                        in0=YB[:, b * SL : b * SL + 1022],
                        in1=YB[:, b * SL + 2 : b * SL + 1024],
                        op=add,
                    )
                # PE mules: absorb lane A / lane B ticks into PE stream.
                M = psum.tile([P, 512], f32)
                nc.tensor.matmul(
                    M[:, 0:2], TG[:, 0:128], CGB[:, 0:2], start=True, stop=True
                )
                M = psum.tile([P, 512], f32)
                nc.tensor.matmul(
                    M[:, 0:2],
                    TG[:, 0:128],
                    CGB[:, 8 * SL : 8 * SL + 2],
                    start=True,
                    stop=True,
                )
                for b in range(NB):
                    t_off = 0 if b < 8 else 256
                    g_off = 128 if b < 8 else 384
                    for h in range(2):
                        cg0 = b * SL + h * 512
                        M = psum.tile([P, 512], f32)
                        nc.tensor.matmul(
                            M[:],
                            TG[:, t_off : t_off + 128],
                            YB[:, cg0 : cg0 + 512],
                            start=True,
                            stop=False,
                        )
                        nc.tensor.matmul(
                            M[:],
                            TG[:, g_off : g_off + 128],
                            CGB[:, cg0 : cg0 + 512],
                            start=False,
                            stop=True,
                        )
                        c0 = b * SL + 1 + h * HALF
                        moff = 1 - h
                        nc.vector.scalar_tensor_tensor(
                            out=YB[:, c0 : c0 + HALF],
                            in0=TH[:, c0 : c0 + HALF],
                            scalar=0.25,
                            in1=M[:, moff : moff + HALF],
                            op0=mult,
                            op1=add,
                        )
                # ACT mules: absorb lane A, lane B, then DVE (last STT) ticks.
                nc.scalar.copy(out=mra[:], in_=CGB[96:128, 0:4])
                nc.scalar.copy(out=mrb[:], in_=CGB[0:32, HB : HB + 4])
                nc.scalar.copy(out=mrd[:], in_=YB[0:32, 8 * SL + 512 : 8 * SL + 516])
                if not last:
                    # ghost_dn (lane A): CG[127, slab b] <- row0 of block b+1
                    nc.scalar.dma_start(
                        out=CGB[127:128, 0 : 8 * SL], in_=YB[1:2, SL:W]
                    )
                    # ghost_up (lane B): CG[0, slab b] <- row125 of block b-1
                    nc.scalar.dma_start(
                        out=CGB[0:1, SL:W], in_=YB[126:127, 0 : 8 * SL]
                    )

            # --- outputs: one DMA per block, alternating lanes ---
            for b in range(NB):
                rows = RB if b < 8 else NI - RB * 8
                r0 = RB * b
                nc.scalar.dma_start(
                    out=o_ap[r0 : r0 + rows, :].bitcast(f32r),
                    in_=YB[1 : 1 + rows, b * SL + 1 : b * SL + 1 + NI],
                )
    _legalize_waits(nc)
    return nc


def _pack_static():
    T0 = np.zeros((P, P), np.float32)
    for q in range(1, 127):
        for pp in (q - 1, q + 1):
            if 1 <= pp <= 126:
                T0[q, pp] = 0.25
    G0 = np.zeros((P, P), np.float32)
    for q in range(1, 127):
        G0[q, q] = 1.0
    G0[0, 1] = 0.25
    G0[127, 126] = 0.25
    nlast = NI - RB * 8  # 14
    T8 = np.zeros((P, P), np.float32)
    for q in range(1, nlast + 1):
        for pp in (q - 1, q + 1):
            if 1 <= pp <= nlast:
                T8[q, pp] = 0.25
    G8 = np.zeros((P, P), np.float32)
    for q in range(1, nlast + 1):
        G8[q, q] = 1.0
    G8[0, 1] = 0.25
    tg = np.zeros((P, 512), np.float32)
    tg[:, 0:128] = T0
    tg[:, 128:256] = G0
    tg[:, 256:384] = T8
    tg[:, 384:512] = G8
    return tg


def kernel(x, pre, f, mu, k1, k2, k3):
    B = pre.shape[0]
    mu_val = float(np.asarray(mu).reshape(-1)[0])
    tg = _pack_static()
    in_maps = []
    for i in range(B):
        yim = np.asarray(pre[i, 0], np.float32)                    # [1022,1022]
        c = (np.asarray(f[i, 0, 1:-1, 1:-1], np.float32) * np.float32(H * H)) / np.float32(
            4.0 * mu_val
        )
        ypack = np.zeros((P, W), np.float32)
        cgpack = np.zeros((P, W), np.float32)
        for b in range(NB):
            r0 = RB * b
            nr = min(RB, NI - r0)
            cb = b * SL
            ypack[1 : 1 + nr, cb + 1 : cb + 1 + NI] = yim[r0 : r0 + nr]
            cgpack[1 : 1 + nr, cb + 1 : cb + 1 + NI] = c[r0 : r0 + nr]
            if b > 0:
                cgpack[0, cb + 1 : cb + 1 + NI] = yim[r0 - 1]
            if r0 + nr < NI:
                cgpack[127, cb + 1 : cb + 1 + NI] = yim[r0 + nr]
        in_maps.append({"tg": tg, "yp": ypack, "cgp": cgpack})

    nc = _build_program()
    res = bass_utils.run_bass_kernel_spmd(nc, in_maps, core_ids=list(range(B)))
    global _LAST_RESULT
    _LAST_RESULT = res
    out = np.stack([res.results[i]["o"] for i in range(B)], axis=0)
    return out.reshape(B, 1, NI, NI).astype(np.float32)


_LAST_RESULT = None


if __name__ == "__main__":
    rng = np.random.default_rng(0)
    inputs = {
        "x": rng.standard_normal((8, 2, NI, NI)).astype(np.float32),
        "pre": rng.standard_normal((8, 1, NI, NI)).astype(np.float32),
        "f": rng.standard_normal((8, 1, 1024, 1024)).astype(np.float32),
        "mu": np.ones((1,), np.float32),
        "k1": np.zeros((1, 1, 3, 3), np.float32),
        "k2": np.zeros((1, 1, 3, 3), np.float32),
        "k3": np.zeros((1, 1, 3, 3), np.float32),
    }
    out = kernel(**inputs)
    print(out.shape, out.dtype, np.abs(out).max())
